# revision 1
# baseline (speedup 1.0000x reference)
"""Trainium2 Bass kernel for CurriculumActorCritic forward pass.

Data-parallel over 8 NeuronCores: 16 images per core.

Host-side algebraic folding (weights only; all data-dependent work on device):
  - input_colors@W1a = onehot(idx) @ (color_embed@W1a): 16x256 LUT folded into mm1 lhsT
  - pos_row@W1b: only 64 distinct rows -> 8 rows per 512-token chunk appended to
    mm1 contraction (K = 16 colors + 8 rows = 24)
  - feat_in is only consumed by K/V projections -> W2_in folded into Wk/Wv,
    k-bias dropped (softmax shift invariance), v-bias folded into attention out bias
  - output_context = mean(relu(h1_out)) @ W2_out: mean computed via activation
    accum_out during relu eviction, W2_out applied to the 256-dim mean (scaled 1/4096)
  - 1/sqrt(hd) folded into Wq; p/v head MLPs merged into one 512-hidden MLP with
    block-diagonal second layer -> final [17, 16] output per core
"""

import numpy as np
import ml_dtypes

import concourse.bass as bass
import concourse.tile as tile
from concourse import bacc
from concourse import mybir
from concourse.bass_utils import run_bass_kernel_spmd

F32 = mybir.dt.float32
BF16 = mybir.dt.bfloat16
I32 = mybir.dt.int32

B, Hh, Ww = 128, 64, 64
NCOL, HID, E, D, NH = 16, 256, 32, 64, 4
HD = D // NH
NCORE = 8
BL = B // NCORE          # 16 images per core
T = Hh * Ww              # 4096 tokens per image
CH = 512                 # chunk of tokens
NCH = T // CH            # 8 chunks per image
AF = mybir.ActivationFunctionType
OP = mybir.AluOpType
AX = mybir.AxisListType

_cache = {}


def _bf(x):
    return np.ascontiguousarray(np.asarray(x).astype(ml_dtypes.bfloat16))


def _f(x):
    return np.ascontiguousarray(np.asarray(x).astype(np.float32))


def build_nc():
    nc = bacc.Bacc("TRN2", target_bir_lowering=False, debug=False, num_devices=NCORE)

    def din(name, shape, dtype):
        return nc.dram_tensor(name, list(shape), dtype, kind="ExternalInput")

    obs_i = din("obs_i", (BL, 3, T), I32)
    L1in = din("L1in", (24, NCH * 256), BF16)
    L1out = din("L1out", (24, NCH * 256), BF16)
    b1in = din("b1in", (128, 2), F32)
    b1out = din("b1out", (128, 2), F32)
    W2KV = din("W2KV", (128, 256), BF16)
    W2o = din("W2o", (128, 128), BF16)
    b2o = din("b2o", (64, 1), F32)
    Wq = din("Wq", (64, 64), BF16)
    bq = din("bq", (64, 1), F32)
    Wo = din("Wo", (64, 64), BF16)
    oab = din("oab", (64, 1), F32)
    rowe = din("rowe", (64, 32), BF16)
    cole = din("cole", (64, 32), BF16)
    pvW1A = din("pvW1A", (128, 512), BF16)
    pvW1B = din("pvW1B", (64, 512), BF16)
    pvb1 = din("pvb1", (128, 4), F32)
    pvW2 = din("pvW2", (128, 68), BF16)
    pvb2 = din("pvb2", (17, 1), F32)
    iota16 = din("iota16", (16, 1), F32)
    iota64 = din("iota64", (64, 1), F32)
    RAMP = din("RAMP", (16, T), F32)
    RAMPI = din("RAMPI", (16, T), F32)
    obs_r = din("obs_r", (BL, 2, 16, T), I32)
    ONEROW = din("ONEROW", (8, CH), BF16)
    HMASK = din("HMASK", (64, 4), F32)

    out_d = nc.dram_tensor("out", [17, BL], F32, kind="ExternalOutput")
    scr_d = nc.dram_tensor("scr", [1, 48], F32, kind="Internal")

    def bc(ap, n):
        return bass.AP(ap.tensor, ap.offset, [[0, n]] + list(ap.ap))

    with tile.TileContext(nc) as tc:
        with (
            tc.tile_pool(name="const", bufs=1) as cp,
            tc.tile_pool(name="big", bufs=1) as bigp,
            tc.tile_pool(name="small", bufs=2) as sp,
        ):
            def csb(t, shape, dtype):
                s = cp.tile(list(shape), dtype, tag=t.name)
                nc.gpsimd.dma_start(s[:], t[:])
                return s

            L1in_s = csb(L1in, (24, NCH * 256), BF16)
            L1out_s = csb(L1out, (24, NCH * 256), BF16)
            b1in_s = csb(b1in, (128, 2), F32)
            b1out_s = csb(b1out, (128, 2), F32)
            W2KV_s = csb(W2KV, (128, 256), BF16)
            W2o_s = csb(W2o, (128, 128), BF16)
            b2o_s = csb(b2o, (64, 1), F32)
            Wq_s = csb(Wq, (64, 64), BF16)
            bq_s = csb(bq, (64, 1), F32)
            Wo_s = csb(Wo, (64, 64), BF16)
            oab_s = csb(oab, (64, 1), F32)
            rowe_s = csb(rowe, (64, 32), BF16)
            cole_s = csb(cole, (64, 32), BF16)
            pvW1A_s = csb(pvW1A, (128, 512), BF16)
            pvW1B_s = csb(pvW1B, (64, 512), BF16)
            pvb1_s = csb(pvb1, (128, 4), F32)
            pvW2_s = csb(pvW2, (128, 68), BF16)
            pvb2_s = csb(pvb2, (17, 1), F32)
            i16_s = csb(iota16, (16, 1), F32)
            i64_s = csb(iota64, (64, 1), F32)
            hm_s = csb(HMASK, (64, 4), F32)
            onerow_s = csb(ONEROW, (8, CH), BF16)

            probe_t = bigp.tile([1, 8], F32)

            def probe(*aps):
                # tiny DVE copies absorb cross-engine sem waits so a following
                # TensorTensor (single HW wait slot) needs at most one wait
                for a in aps:
                    nc.vector.tensor_scalar(probe_t[0:1, 0:1], a[0:1, 0:1], 1.0, None, OP.mult)

            ctx_acc = bigp.tile([128, 256], F32)
            blk_all = bigp.tile([64, 4 * BL], BF16)
            scores = bigp.tile([64, T], F32)
            qhT = bigp.tile([64, BL], F32)
            curb = bigp.tile([64, BL], BF16)
            cA = bigp.tile([128, BL], F32)
            att_f = bigp.tile([64, BL], F32)

            # ---- phase 0: mask scan -> cur, q ----
            with (
                tc.tile_pool(name="p0", bufs=1) as p0,
                tc.tile_pool(name="ps0", bufs=2, space="PSUM") as pssm,
            ):
                mask_s = p0.tile([16, T], I32, tag="mask")
                nc.gpsimd.dma_start(mask_s[:], obs_i[:, 2, :])
                ramp_s = p0.tile([16, T], F32, tag="ramp")
                nc.gpsimd.dma_start(ramp_s[:], RAMP[:, :])

                mnz = p0.tile([16, T], F32, tag="mnz")
                nc.vector.tensor_scalar(mnz[:], mask_s[:], 0, None, OP.not_equal)
                sc16 = sp.tile([16, 8], F32, tag="sc16")
                val = p0.tile([16, T], F32, tag="val")
                probe(mnz, ramp_s)
                nc.vector.tensor_tensor(val[:], mnz[:], ramp_s[:], OP.mult)
                nc.vector.tensor_reduce(sc16[:, 0:1], val[:], AX.X, OP.max)
                nc.vector.tensor_scalar(sc16[:, 1:2], sc16[:, 0:1], -1.0, 4096.0, OP.mult, OP.add)
                rampi_s = p0.tile([16, T], F32, tag="ramp")
                nc.gpsimd.dma_start(rampi_s[:], RAMPI[:, :])
                val2 = p0.tile([16, T], F32, tag="val")
                probe(mnz, rampi_s)
                nc.vector.tensor_tensor(val2[:], mnz[:], rampi_s[:], OP.mult)
                nc.vector.tensor_reduce(sc16[:, 2:3], val2[:], AX.X, OP.max)
                nc.vector.tensor_scalar(sc16[:, 3:4], sc16[:, 2:3], -1.0, 64.0, OP.mult, OP.add)
                nc.vector.tensor_scalar(sc16[:, 4:5], sc16[:, 3:4], -64.0, None, OP.mult)
                probe(sc16)
                nc.vector.tensor_tensor(sc16[:, 5:6], sc16[:, 1:2], sc16[:, 4:5], OP.add)
                nc.vector.tensor_scalar(sc16[:, 6:7], sc16[:, 0:1], 0.0, None, OP.is_gt)

                nc.gpsimd.dma_start(scr_d[0, 0:16], sc16[:, 3:4])
                nc.gpsimd.dma_start(scr_d[0, 16:32], sc16[:, 5:6])
                nc.gpsimd.dma_start(scr_d[0, 32:48], sc16[:, 6:7])
                rep3 = sp.tile([64, 3 * 16], F32, tag="rep3")
                nc.gpsimd.dma_start(rep3[:, :], bc(scr_d[0, :], 64))

                ohi = sp.tile([64, 16], BF16, tag="ohi")
                nc.vector.tensor_scalar(ohi[:], rep3[:, 0:16], i64_s[:, 0:1], None, OP.is_equal)
                ohj = sp.tile([64, 16], BF16, tag="ohj")
                nc.vector.tensor_scalar(ohj[:], rep3[:, 16:32], i64_s[:, 0:1], None, OP.is_equal)

                curps_r = pssm.tile([32, 16], F32, tag="ps")
                nc.tensor.matmul(curps_r[:], rowe_s[:], ohi[:], start=True, stop=True)
                curps_c = pssm.tile([32, 16], F32, tag="ps")
                nc.tensor.matmul(curps_c[:], cole_s[:], ohj[:], start=True, stop=True)
                curT = sp.tile([64, 16], F32, tag="curT")
                nc.scalar.activation(curT[0:32, :], curps_r[:], AF.Copy)
                nc.scalar.activation(curT[32:64, :], curps_c[:], AF.Copy)
                curM = sp.tile([64, 16], F32, tag="curM")
                probe(curT, rep3)
                nc.vector.tensor_tensor(curM[:], curT[:], rep3[:, 32:48], OP.mult)
                nc.vector.tensor_copy(curb[:], curM[:])

                qps = pssm.tile([64, 16], F32, tag="ps")
                nc.tensor.matmul(qps[:], Wq_s[:], curb[:], start=True, stop=True)
                nc.vector.tensor_scalar(qhT[:], qps[:], bq_s[:, 0:1], None, OP.add)
                probe(hm_s, qhT)
                for b in range(BL):
                    q_ap = qhT[:, b:b + 1]
                    q_bc = bass.AP(q_ap.tensor, q_ap.offset, [list(q_ap.ap[0]), [0, 4]])
                    nc.vector.tensor_tensor(blk_all[:, 4 * b:4 * b + 4], q_bc, hm_s[:], OP.mult)

            # ---- phase 1: big token loop ----
            v_all = bigp.tile([128, BL * T // 2], BF16)  # chunk G: half G%2, cols (G//2)*CH
            with (
                tc.tile_pool(name="r1", bufs=4) as r1p,
                tc.tile_pool(name="oh", bufs=3) as ohp,
                tc.tile_pool(name="idx", bufs=3) as idxp,
                tc.tile_pool(name="ps1", bufs=3, space="PSUM") as ps1,
                tc.tile_pool(name="pskv", bufs=2, space="PSUM") as pskv,
            ):
                # in-grid: c outer so 16 batches' score rows share one psum bank
                for c in range(NCH):
                    for b in range(BL):
                        idx_rep = idxp.tile([16, CH], I32)
                        nc.gpsimd.dma_start(
                            idx_rep[:], obs_r[b, 0, :, c * CH:(c + 1) * CH])
                        oh = ohp.tile([24, CH], BF16)
                        nc.gpsimd.dma_start(oh[16:24, :], onerow_s[:])
                        nc.vector.tensor_scalar(
                            oh[0:16, :], idx_rep[:], i16_s[:, 0:1], None, OP.is_equal)
                        r1 = []
                        for m in range(2):
                            ps = ps1.tile([128, CH], F32)
                            nc.tensor.matmul(
                                ps[:],
                                L1in_s[:, c * 256 + m * 128: c * 256 + (m + 1) * 128],
                                oh[:], start=True, stop=True)
                            r1m = r1p.tile([128, CH], BF16)
                            nc.scalar.activation(
                                r1m[:], ps[:], AF.Relu, bias=b1in_s[:, m:m + 1])
                            r1.append(r1m)
                        kvps = pskv.tile([128, CH], F32)
                        nc.tensor.matmul(kvps[:], W2KV_s[:, 0:128], r1[0][:],
                                         start=True, stop=False)
                        nc.tensor.matmul(kvps[:], W2KV_s[:, 128:256], r1[1][:],
                                         start=False, stop=True)
                        G = b * 8 + c
                        hf = (G % 2) * 64
                        sl = (G // 2) * CH
                        nc.vector.tensor_copy(
                            v_all[hf:hf + 64, sl:sl + CH], kvps[64:128, :])
                        k_ch = r1p.tile([64, CH], BF16, tag="kch")
                        nc.vector.tensor_copy(k_ch[:], kvps[0:64, :])
                        sps = pskv.tile([4, CH], F32, tag="sps")
                        nc.tensor.matmul(sps[:], blk_all[:, 4 * b:4 * b + 4],
                                         k_ch[:], start=True, stop=True)
                        sb = r1p.tile([4, CH], F32, tag="sb")
                        nc.scalar.activation(sb[:], sps[:], AF.Copy)
                        nc.gpsimd.dma_start(
                            scores[4 * b:4 * b + 4, c * CH:(c + 1) * CH], sb[:])

                # out-grid: only the relu mean is needed
                for b in range(BL):
                    for c in range(NCH):
                        idx_rep = idxp.tile([16, CH], I32)
                        nc.gpsimd.dma_start(
                            idx_rep[:], obs_r[b, 1, :, c * CH:(c + 1) * CH])
                        oh = ohp.tile([24, CH], BF16)
                        nc.gpsimd.dma_start(oh[16:24, :], onerow_s[:])
                        nc.vector.tensor_scalar(
                            oh[0:16, :], idx_rep[:], i16_s[:, 0:1], None, OP.is_equal)
                        for m in range(2):
                            ps = ps1.tile([128, CH], F32)
                            nc.tensor.matmul(
                                ps[:],
                                L1out_s[:, c * 256 + m * 128: c * 256 + (m + 1) * 128],
                                oh[:], start=True, stop=True)
                            r1m = r1p.tile([128, CH], BF16)
                            col = (m * 16 + b) * 8 + c
                            nc.scalar.activation(
                                r1m[:], ps[:], AF.Relu, bias=b1out_s[:, m:m + 1],
                                accum_out=ctx_acc[:, col:col + 1])

            # ---- phase 2: ctx mm, attention, head ----
            pssm = ctx2 = tc.tile_pool(name="ps2", bufs=2, space="PSUM")
            pssm = pssm.__enter__()
            red = sp.tile([128, 32], F32, tag="red")
            nc.vector.tensor_reduce(
                red[:], ctx_acc[:].rearrange("p (g c) -> p g c", c=8), AX.X, OP.add)
            redb = sp.tile([128, 32], BF16, tag="redb")
            nc.vector.tensor_copy(redb[:], red[:])
            ctxps = pssm.tile([64, 16], F32, tag="ps")
            nc.tensor.matmul(ctxps[:], W2o_s[:, 0:64], redb[:, 0:16], start=True, stop=False)
            nc.tensor.matmul(ctxps[:], W2o_s[:, 64:128], redb[:, 16:32], start=False, stop=True)
            nc.vector.tensor_scalar(cA[64:128, :], ctxps[:], b2o_s[:, 0:1], None, OP.add)

            smax = sp.tile([64, 4], F32, tag="smax")
            nc.vector.tensor_reduce(smax[:, 0:1], scores[:], AX.X, OP.max)
            nc.vector.tensor_scalar(smax[:, 1:2], smax[:, 0:1], -1.0, None, OP.mult)
            attn_f = bigp.tile([64, T], F32)
            nc.scalar.activation(attn_f[:], scores[:], AF.Exp, bias=smax[:, 1:2],
                                 accum_out=smax[:, 2:3])
            nc.vector.reciprocal(smax[:, 3:4], smax[:, 2:3])
            attn_n = bigp.tile([64, T], BF16)
            nc.vector.tensor_scalar(attn_n[:], attn_f[:], smax[:, 3:4], None, OP.mult)

            with tc.tile_pool(name="arep", bufs=2) as arp:
                for b in range(BL):
                    arep = arp.tile([64, T], BF16, tag="arep")
                    src = attn_n[4 * b:4 * b + 4, :]
                    for d in range(16):
                        dst = arep[:]
                        dst_ap = bass.AP(dst.tensor, dst.offset + d * dst.ap[0][0],
                                         [[16 * dst.ap[0][0], 4], [1, T]])
                        nc.gpsimd.dma_start(dst_ap, src)
                    vg = arp.tile([64, T], BF16, tag="vg")
                    for c in range(NCH):
                        G = b * 8 + c
                        hf = (G % 2) * 64
                        sl = (G // 2) * CH
                        nc.gpsimd.dma_start(vg[:, c * CH:(c + 1) * CH],
                                            v_all[hf:hf + 64, sl:sl + CH])
                    tmp = arp.tile([64, T], BF16, tag="atmp")
                    probe(arep, vg)
                    nc.vector.tensor_tensor(tmp[:], arep[:], vg[:], OP.mult)
                    nc.vector.tensor_reduce(att_f[:, b:b + 1], tmp[:], AX.X, OP.add)

            attb = sp.tile([64, 16], BF16, tag="attb")
            nc.vector.tensor_copy(attb[:], att_f[:])
            aps = pssm.tile([64, 16], F32, tag="ps")
            nc.tensor.matmul(aps[:], Wo_s[:], attb[:], start=True, stop=True)
            nc.vector.tensor_scalar(cA[0:64, :], aps[:], oab_s[:, 0:1], None, OP.add)

            cAb = sp.tile([128, 16], BF16, tag="cAb")
            nc.vector.tensor_copy(cAb[:], cA[:])
            hpv = sp.tile([128, 64], BF16, tag="hpv")
            for m in range(4):
                pps = pssm.tile([128, 16], F32, tag="ps")
                nc.tensor.matmul(pps[:], pvW1A_s[:, m * 128:(m + 1) * 128], cAb[:],
                                 start=True, stop=False)
                nc.tensor.matmul(pps[:], pvW1B_s[:, m * 128:(m + 1) * 128], curb[:],
                                 start=False, stop=True)
                nc.scalar.activation(hpv[:, m * 16:(m + 1) * 16], pps[:], AF.Relu,
                                     bias=pvb1_s[:, m:m + 1])
            ops = pssm.tile([17, 16], F32, tag="ps")
            for m in range(4):
                nc.tensor.matmul(ops[:], pvW2_s[:, m * 17:(m + 1) * 17],
                                 hpv[:, m * 16:(m + 1) * 16],
                                 start=(m == 0), stop=(m == 3))
            outT = sp.tile([17, 16], F32, tag="outT")
            nc.vector.tensor_scalar(outT[:], ops[:], pvb2_s[:, 0:1], None, OP.add)
            nc.gpsimd.dma_start(out_d[:], outT[:])
            ctx2.__exit__(None, None, None)

    nc.compile()
    return nc


def prep_consts(inp):
    t = np.arange(T, dtype=np.float32)
    c = {
        "b1in": _f(inp["in_b1"].reshape(2, 128).T),
        "b1out": _f(inp["out_b1"].reshape(2, 128).T),
        "b2o": _f(inp["out_b2"].reshape(64, 1)),
        "bq": _f((inp["bq"] * 0.25).reshape(64, 1)),
        "Wq": _bf(inp["Wq"] * 0.25),
        "Wo": _bf(inp["Wo"]),
        "rowe": _bf(inp["row_embed"]),
        "cole": _bf(inp["col_embed"]),
        "pvb2": _f(np.concatenate([inp["p_b2"], inp["v_b2"]]).reshape(17, 1)),
        "iota16": np.arange(16, dtype=np.float32).reshape(16, 1),
        "iota64": np.arange(64, dtype=np.float32).reshape(64, 1),
        "RAMP": _f(np.broadcast_to((T - t).reshape(1, T), (16, T))),
        "RAMPI": _f(np.broadcast_to((64 - t // 64).reshape(1, T), (16, T))),
        "ONEROW": _bf((np.arange(CH) // 64 == np.arange(8)[:, None]).astype(np.float32)),
        "HMASK": _f((np.arange(64)[:, None] // 16 == np.arange(4)[None, :]).astype(np.float32)),
    }
    for pre, wk in (("L1in", "in_W1"), ("L1out", "out_W1")):
        W1 = inp[wk]
        CW1 = inp["color_embed"] @ W1[:32]
        PW1 = inp["row_embed"] @ W1[32:64]
        v = np.stack([np.concatenate([CW1, PW1[8 * cc:8 * cc + 8]], 0) for cc in range(NCH)])
        c[pre] = _bf(v.transpose(1, 0, 2).reshape(24, NCH * 256))
    W2k = inp["in_W2"] @ inp["Wk"]
    W2v = inp["in_W2"] @ inp["Wv"]
    W2KV = np.concatenate([W2k, W2v], 1)
    c["W2KV"] = _bf(W2KV.reshape(2, 128, 128).transpose(1, 0, 2).reshape(128, 256))
    W2o = inp["out_W2"] / T
    c["W2o"] = _bf(W2o.reshape(2, 128, 64).transpose(1, 0, 2).reshape(128, 128))
    c["oab"] = _f(((inp["in_b2"] @ inp["Wv"] + inp["bv"]) @ inp["Wo"] + inp["bo"]).reshape(64, 1))
    pvW1 = np.concatenate([inp["p_W1"], inp["v_W1"]], 1)
    c["pvW1A"] = _bf(pvW1[:128])
    c["pvW1B"] = _bf(pvW1[128:192])
    pvb1 = np.concatenate([inp["p_b1"], inp["v_b1"]])
    c["pvb1"] = _f(pvb1.reshape(4, 128).T)
    pvW2 = np.zeros((512, 17), np.float32)
    pvW2[:256, :16] = inp["p_W2"]
    pvW2[256:, 16:] = inp["v_W2"]
    c["pvW2"] = _bf(pvW2.reshape(4, 128, 17).transpose(1, 0, 2).reshape(128, 68))
    return c


def kernel(**inputs):
    if "nc" not in _cache:
        _cache["nc"] = build_nc()
    nc = _cache["nc"]
    c = prep_consts(inputs)
    obs = np.asarray(inputs["obs"]).reshape(B, 3, T).astype(np.int32)
    in_maps = []
    for i in range(NCORE):
        m = dict(c)
        ob = obs[i * BL:(i + 1) * BL]
        m["obs_i"] = np.ascontiguousarray(ob)
        m["obs_r"] = np.ascontiguousarray(
            np.broadcast_to(ob[:, 0:2, None, :], (BL, 2, 16, T)).astype(np.int32))
        in_maps.append(m)
    res = run_bass_kernel_spmd(nc, in_maps, list(range(NCORE)))
    out = np.concatenate([r["out"].T for r in res.results], 0)
    return out.astype(np.float32)

